# revision 1
# baseline (speedup 1.0000x reference)
"""Trainium2 Bass kernel for nn_AttentionLayer (B=8, S=1024, D=1024, H=16, HD=64).

Strategy: pure data parallelism — one batch element per NeuronCore (8 cores).
Weights are replicated (pre-transposed on host so the contraction dim lands on
SBUF partitions); x is sharded on batch and pre-transposed per shard.

Per-core compute layout (all transposes eliminated by construction):
  Qt/Kt [dout, s]  = W^T-stacked @ x^T         (d on partitions)
  Vx    [s, dout]  natural, 65-strided per head with a ones column; padded
                    keys' rows are zeroed (this IS the pad mask: they then
                    contribute 0 to both attention output and denominator)
  scoresT[k, q]    = Kt_h^T @ Qt_h             (k on partitions, q free);
                    even/odd heads sit at partition bases 0/64, so the two
                    K=64 matmuls of a head pair run concurrently on the PE
                    (disjoint row groups). Causal masking of the diagonal
                    256-col window = identity-matmul accumulating a -1e9
                    triangle mask into the scores PSUM.
  expT   [k, q]    = exp(scoresT / 8)          (ACT, PSUM->SBUF, bf16 out)
  avT -> out[q, d] via lhsT=[V_h | 1]: ones column also produces the softmax
                    denominator as psum row 64; accumulated per 512-wide
                    q-chunk so a head pair needs 4 PSUM banks.
  attn_outT [dmid, s] assembled via SBUF->SBUF DMA, normalized by 1/denom
                    (per-pair DRAM bounce + partition-broadcast DMA), then
  out [s, dout]    = attn_outT^T @ Wo^T + bo   (bias via K=1 matmul)

Matmul dtype: bf16 (f32r would be full fp32 precision at the same PE rate,
but hangs TRN2 hardware - observed empirically). End-to-end rel err vs the
fp32 reference is ~4e-3.
"""

import os
import sys
import types

import numpy as np

B, S, D, H, HD = 8, 1024, 1024, 16, 64
NT = D // 128          # 8 partition tiles
PAD_ID = 1.0
NEG = -1e9
SCALE = 1.0 / 8.0      # 1/sqrt(HD)

MM_MODE = os.environ.get("KERNEL_MM_MODE", "bf16")

_CACHE = {}
LAST_RESULT = None
LAST_EXEC_NS = None


def _install_trace_hook():
    """Provide antenv.axon_hooks (missing in this image) so trace=True works."""
    try:
        import antenv
        if "antenv.axon_hooks" in sys.modules:
            return True
        m = types.ModuleType("antenv.axon_hooks")
        _hook = [None]
        m.set_axon_ntff_profile_hook = lambda h: _hook.__setitem__(0, h)
        m.get_axon_ntff_profile_hook = lambda: _hook[0]
        sys.modules["antenv.axon_hooks"] = m
        antenv.axon_hooks = m
        from trn_agent_boot.trn_boot import _ntff_profile_via_ctypes
        hook = _ntff_profile_via_ctypes("/opt/axon/libaxon_pjrt.so")
        if hook is None:
            return False
        m.set_axon_ntff_profile_hook(hook)
        return True
    except Exception:
        return False


def _build_graph():
    import concourse.bass as bass
    import concourse.mybir as mybir
    import concourse.tile as tile
    from concourse import bacc

    F32 = mybir.dt.float32
    MMD = {"bf16": mybir.dt.bfloat16, "f32r": mybir.dt.float32r,
           "f32": mybir.dt.float32}[MM_MODE]
    AluOp = mybir.AluOpType
    Act = mybir.ActivationFunctionType

    nc = bacc.Bacc(target_bir_lowering=False)

    xT = nc.declare_dram_parameter("xT", [D, S], MMD, isOutput=False)
    WqT = nc.declare_dram_parameter("WqT", [D, D], MMD, isOutput=False)
    WkT = nc.declare_dram_parameter("WkT", [D, D], MMD, isOutput=False)
    WvT = nc.declare_dram_parameter("WvT", [D, D], MMD, isOutput=False)
    WoT = nc.declare_dram_parameter("WoT", [D, D], MMD, isOutput=False)
    bv = nc.declare_dram_parameter("bv", [D], MMD, isOutput=False)
    bo = nc.declare_dram_parameter("bo", [D], MMD, isOutput=False)
    ones_p = nc.declare_dram_parameter("ones", [S], MMD, isOutput=False)
    # smalls: [128, 24] f32 = ids_r | bq_r | bk_r (each [128, 8], host-packed)
    smalls = nc.declare_dram_parameter("smalls", [128, 3 * NT], F32, isOutput=False)
    # causal masks [128, 512] = maskA (even chunks) | maskB (odd chunks)
    masks_p = nc.declare_dram_parameter("masks", [128, 512], MMD, isOutput=False)
    ident_p = nc.declare_dram_parameter("ident", [128, 128], MMD, isOutput=False)
    out_e = nc.declare_dram_parameter("out", [S, D], F32, isOutput=True)

    denom_d = nc.dram_tensor("denom_d", [H, S], MMD)
    recip_d = nc.dram_tensor("recip_d", [H, S], MMD)

    def dep(later, earlier, reason):
        bass._add_dep_helper(later.ins, earlier.ins, reason=reason)

    with tile.TileContext(nc) as tc:
        with tc.tile_pool(name="const", bufs=1) as cp, \
             tc.tile_pool(name="qkv", bufs=1) as qp:

            # ---- constants ----
            sm = cp.tile([128, 3 * NT], F32, tag="sm", name="sm")
            nc.sync.dma_start(out=sm[:], in_=smalls[:])
            pad01 = cp.tile([128, NT], F32, tag="pad01", name="pad01")
            nc.vector.tensor_scalar(out=pad01[:], in0=sm[:, 0:NT],
                                    scalar1=PAD_ID, scalar2=None,
                                    op0=AluOp.not_equal)
            bq_col = sm[:, NT:2 * NT]
            bk_col = sm[:, 2 * NT:3 * NT]
            bv_row = cp.tile([1, D], MMD, tag="bvr", name="bv_row")
            nc.sync.dma_start(out=bv_row[:], in_=bv[None, :])
            bo_row = cp.tile([1, D], MMD, tag="bor", name="bo_row")
            nc.sync.dma_start(out=bo_row[:], in_=bo[None, :])
            ones_row = cp.tile([1, S], MMD, tag="ones", name="ones_row")
            nc.sync.dma_start(out=ones_row[:], in_=ones_p[None, :])
            masks_sb = cp.tile([128, 512], MMD, tag="masks", name="masks_sb")
            nc.sync.dma_start(out=masks_sb[:], in_=masks_p[:])
            ident = cp.tile([128, 128], MMD, tag="ident", name="ident")
            nc.sync.dma_start(out=ident[:], in_=ident_p[:])

            # ---- persistent per-core tensors ----
            Qt = [qp.tile([128, S], MMD, tag=f"qt{t}", name=f"qt{t}")
                  for t in range(NT)]
            Kt = [qp.tile([128, S], MMD, tag=f"kt{t}", name=f"kt{t}")
                  for t in range(NT)]
            Vx = [qp.tile([128, H * (HD + 1)], MMD, tag=f"vx{t}", name=f"vx{t}")
                  for t in range(NT)]

            # ============ Phase B: projections ============
            # V first, then Q/K interleaved per output tile, so attention
            # head-pair t unblocks as soon as Qt[t]/Kt[t] land (instead of
            # after the whole K projection) and the PE never drains across
            # the phase boundary.
            with tc.tile_pool(name="xw", bufs=1) as xp, \
                 tc.tile_pool(name="wst", bufs=8) as wp, \
                 tc.tile_pool(name="wqk", bufs=16) as wqkp, \
                 tc.tile_pool(name="psp", bufs=4, space="PSUM") as pp:

                xT_sb = [xp.tile([128, S], MMD, tag=f"x{c}", name=f"x{c}")
                         for c in range(NT)]
                for c in range(NT):
                    nc.sync.dma_start(out=xT_sb[c][:], in_=xT[c * 128:(c + 1) * 128, :])

                def stream_w(w_ext, pool):
                    tiles = []
                    for c in range(NT):
                        t = pool.tile([128, D], MMD, tag="w", name="w_t")
                        nc.sync.dma_start(out=t[:], in_=w_ext[c * 128:(c + 1) * 128, :])
                        tiles.append(t)
                    return tiles

                w_sb = stream_w(WvT, wp)
                for m in range(NT):
                    vdst = Vx[m][:].rearrange("p (h e) -> p h e", e=HD + 1)
                    nc.vector.memset(vdst[:, :, HD:HD + 1], 1.0)
                    for n in range(2):
                        ps = pp.tile([128, 512], F32, tag="pp", name="ps")
                        for c in range(NT):
                            nc.tensor.matmul(
                                ps[:],
                                xT_sb[c][:, m * 128:(m + 1) * 128],
                                w_sb[c][:, n * 512:(n + 1) * 512],
                                start=(c == 0), stop=False)
                        nc.tensor.matmul(ps[:], ones_row[:, :128],
                                         bv_row[:, n * 512:(n + 1) * 512],
                                         start=False, stop=True)
                        nc.vector.tensor_copy(
                            out=vdst[:, n * 8:(n + 1) * 8, 0:HD],
                            in_=ps[:].rearrange("p (h e) -> p h e", e=HD))
                    # pad mask: zero whole rows (keys) where ids == PAD,
                    # including the ones column -> denominator excludes them
                    nc.vector.tensor_scalar(
                        out=Vx[m][:], in0=Vx[m][:],
                        scalar1=pad01[:, m:m + 1], scalar2=None,
                        op0=AluOp.mult)

                wq_sb = stream_w(WqT, wqkp)
                wk_sb = stream_w(WkT, wqkp)
                for m in range(NT):
                    for w_sb2, dst, bias_col in ((wq_sb, Qt, bq_col),
                                                 (wk_sb, Kt, bk_col)):
                        for n in range(2):
                            ps = pp.tile([128, 512], F32, tag="pp", name="ps")
                            for c in range(NT):
                                nc.tensor.matmul(
                                    ps[:],
                                    w_sb2[c][:, m * 128:(m + 1) * 128],
                                    xT_sb[c][:, n * 512:(n + 1) * 512],
                                    start=(c == 0), stop=(c == NT - 1))
                            nc.vector.tensor_scalar(
                                out=dst[m][:, n * 512:(n + 1) * 512], in0=ps[:],
                                scalar1=bias_col[:, m:m + 1], scalar2=None,
                                op0=AluOp.add)

            # ============ Phase C: attention (head pairs) ============
            with tc.tile_pool(name="aot", bufs=1) as ap_pool, \
                 tc.tile_pool(name="wo", bufs=8) as wop:
              aoT = [ap_pool.tile([128, S], MMD, tag=f"ao{t}", name=f"ao{t}")
                     for t in range(NT)]
              with tc.tile_pool(name="expp", bufs=6) as ep, \
                   tc.tile_pool(name="avst", bufs=3) as avs, \
                   tc.tile_pool(name="dn", bufs=2) as dnp, \
                   tc.tile_pool(name="rcp", bufs=2) as rcpp, \
                   tc.tile_pool(name="pssc", bufs=2, space="PSUM") as psc, \
                   tc.tile_pool(name="psav", bufs=1, space="PSUM") as pav:

                # prefetch Wo during attention
                wo_sb = []
                for c in range(NT):
                    w_t = wop.tile([128, D], MMD, tag="wo", name="wo_t")
                    nc.sync.dma_start(out=w_t[:], in_=WoT[c * 128:(c + 1) * 128, :])
                    wo_sb.append(w_t)

                denom_writes = {}
                for t in range(NT):         # head pair (2t, 2t+1)
                    av_ps = {(par, g): pav.tile([HD + 1, 512], F32,
                                                tag=f"av{par}{g}",
                                                name=f"av{par}{g}")
                             for par in range(2) for g in range(2)}
                    ex_t = {}
                    for c in range(NT):
                        qs = 256 * (c // 2)
                        cols = S - qs
                        for par in range(2):
                            base = par * 64
                            sc = psc.tile([128, 1024], F32, tag="sc", name="sc")
                            nchunks = list(range(qs, S, 512))
                            for n0 in nchunks:
                                w = min(512, S - n0)
                                nc.tensor.matmul(
                                    sc[:, n0 - qs:n0 - qs + w],
                                    Kt[t][base:base + 64, c * 128:(c + 1) * 128],
                                    Qt[t][base:base + 64, n0:n0 + w],
                                    start=True, stop=not (n0 == qs))
                            # causal mask for the diagonal window via
                            # identity-matmul accumulation (bank A group)
                            nc.tensor.matmul(
                                sc[:, 0:256], ident[:],
                                masks_sb[:, 256 * (c % 2):256 * (c % 2) + 256],
                                start=False, stop=True)
                            ex = ep.tile([128, 1024], MMD, tag="ex",
                                         name="ex")
                            ex_t[par] = ex
                            if qs % 512:
                                # zero the gap so 512-aligned avT reads are valid
                                nc.vector.memset(ex[:, qs - 256:qs], 0.0)
                            nc.scalar.activation(out=ex[:, qs:S],
                                                 in_=sc[:, 0:cols],
                                                 func=Act.Exp, scale=SCALE)
                            for g in range(2):
                                if c <= 4 * g + 3:
                                    nc.tensor.matmul(
                                        av_ps[(par, g)][:],
                                        Vx[c][:, (2 * t + par) * (HD + 1):
                                              (2 * t + par + 1) * (HD + 1)],
                                        ex[:, 512 * g:512 * (g + 1)],
                                        start=(c == 0),
                                        stop=(c == min(4 * g + 3, NT - 1)))
                    for par in range(2):
                        h = 2 * t + par
                        st = avs.tile([HD + 1, S], MMD, tag="st", name="st")
                        for g in range(2):
                            nc.vector.tensor_copy(
                                out=st[:, 512 * g:512 * (g + 1)],
                                in_=av_ps[(par, g)][:])
                        nc.sync.dma_start(
                            out=aoT[t][par * 64:par * 64 + HD, :],
                            in_=st[0:HD, :])
                        dw = nc.sync.dma_start(
                            out=denom_d[h:h + 1, :], in_=st[HD:HD + 1, :])
                        denom_writes[h] = dw

                    # per-pair denominator -> reciprocal -> broadcast -> norm
                    d_sb = dnp.tile([128, 2, NT], MMD, tag="dsb", name="d_sb")
                    rd = nc.sync.dma_start(
                        out=d_sb[:],
                        in_=denom_d[2 * t:2 * t + 2, :].rearrange(
                            "h (g p) -> p h g", p=128))
                    dep(rd, denom_writes[2 * t], reason="denom RAW via DRAM")
                    dep(rd, denom_writes[2 * t + 1], reason="denom RAW via DRAM")
                    r_sb = dnp.tile([128, 2, NT], MMD, tag="rsb", name="r_sb")
                    with nc.allow_low_precision(
                            reason="softmax recip in compute dtype; "
                                   "error budget verified offline"):
                        nc.vector.reciprocal(out=r_sb[:], in_=d_sb[:])
                    wr = nc.sync.dma_start(
                        out=recip_d[2 * t:2 * t + 2, :].rearrange(
                            "h (g p) -> p h g", p=128),
                        in_=r_sb[:])
                    rec = rcpp.tile([128, S], MMD, tag="rec", name="rec")
                    b0 = nc.sync.dma_start(
                        out=rec[0:64, :],
                        in_=recip_d[2 * t, :][None, :].broadcast_to([64, S]))
                    b1 = nc.sync.dma_start(
                        out=rec[64:128, :],
                        in_=recip_d[2 * t + 1, :][None, :].broadcast_to([64, S]))
                    dep(b0, wr, reason="recip RAW via DRAM")
                    dep(b1, wr, reason="recip RAW via DRAM")
                    nc.vector.tensor_mul(aoT[t][:], aoT[t][:], rec[:])

              # ============ Phase E: output projection ============
              with tc.tile_pool(name="ost", bufs=3) as osp, \
                   tc.tile_pool(name="psf", bufs=4, space="PSUM") as pf:
                for m in range(NT):
                    for n in range(2):
                        ps = pf.tile([128, 512], F32, tag="pf", name="psf")
                        for c in range(NT):
                            nc.tensor.matmul(
                                ps[:],
                                aoT[c][:, m * 128:(m + 1) * 128],
                                wo_sb[c][:, n * 512:(n + 1) * 512],
                                start=(c == 0), stop=False)
                        nc.tensor.matmul(ps[:], ones_row[:, :128],
                                         bo_row[:, n * 512:(n + 1) * 512],
                                         start=False, stop=True)
                        ot = osp.tile([128, 512], F32, tag="ot", name="ot")
                        nc.scalar.copy(out=ot[:], in_=ps[:])
                        nc.sync.dma_start(
                            out=out_e[m * 128:(m + 1) * 128, n * 512:(n + 1) * 512],
                            in_=ot[:])
    nc.finalize()
    return nc


def _np_mm_dtype():
    if MM_MODE == "bf16":
        import ml_dtypes
        return ml_dtypes.bfloat16
    return np.float32


def _host_consts(mmdt):
    jj = np.arange(256)[None, :]
    pp = np.arange(128)[:, None]
    maskA = np.where((jj < 128) & (jj < pp), NEG, 0.0)
    maskB = np.where((jj < 128) | (jj - 128 < pp), NEG, 0.0)
    masks = np.concatenate([maskA, maskB], axis=1).astype(mmdt)
    ident = np.eye(128, dtype=np.float32).astype(mmdt)
    return masks, ident


def build_in_maps(x, input_ids, Wq, bq, Wk, bk, Wv, bv, Wo, bo):
    x = np.asarray(x, dtype=np.float32)
    input_ids = np.asarray(input_ids)
    mmdt = _np_mm_dtype()
    masks, ident = _host_consts(mmdt)
    bq_r = np.ascontiguousarray(np.asarray(bq, np.float32).reshape(NT, 128).T)
    bk_r = np.ascontiguousarray(np.asarray(bk, np.float32).reshape(NT, 128).T)
    shared = {
        "WqT": np.ascontiguousarray(np.asarray(Wq, np.float32).T).astype(mmdt),
        "WkT": np.ascontiguousarray(np.asarray(Wk, np.float32).T).astype(mmdt),
        "WvT": np.ascontiguousarray(np.asarray(Wv, np.float32).T).astype(mmdt),
        "WoT": np.ascontiguousarray(np.asarray(Wo, np.float32).T).astype(mmdt),
        "bv": np.asarray(bv, np.float32).astype(mmdt),
        "bo": np.asarray(bo, np.float32).astype(mmdt),
        "ones": np.ones([S], mmdt),
        "masks": masks, "ident": ident,
    }
    in_maps = []
    for b in range(B):
        ids_r = input_ids[b].astype(np.float32).reshape(NT, 128).T
        m = dict(shared)
        m["xT"] = np.ascontiguousarray(x[b].T).astype(mmdt)
        m["smalls"] = np.ascontiguousarray(
            np.concatenate([ids_r, bq_r, bk_r], axis=1)).astype(np.float32)
        in_maps.append(m)
    return in_maps


def kernel(x, input_ids, Wq, bq, Wk, bk, Wv, bv, Wo, bo):
    global LAST_RESULT, LAST_EXEC_NS
    from concourse.bass_utils import run_bass_kernel_spmd

    if "nc" not in _CACHE:
        _CACHE["nc"] = _build_graph()
    nc = _CACHE["nc"]
    in_maps = build_in_maps(x, input_ids, Wq, bq, Wk, bk, Wv, bv, Wo, bo)

    trace = os.environ.get("KERNEL_TRACE", "0") == "1" and _install_trace_hook()
    res = run_bass_kernel_spmd(nc, in_maps, core_ids=list(range(B)), trace=trace)
    LAST_RESULT = res
    LAST_EXEC_NS = res.exec_time_ns
    return np.stack([res.results[b]["out"] for b in range(B)]).astype(np.float32)



# revision 5
# speedup vs baseline: 1.2492x; 1.2492x over previous
"""Trainium2 Bass kernel for nn_AttentionLayer (B=8, S=1024, D=1024, H=16, HD=64).

Strategy: pure data parallelism - one batch element per NeuronCore (8 cores).
Weights replicated (pre-transposed on host so the contraction dim lands on SBUF
partitions); x sharded on batch and pre-transposed per shard.

v2 changes vs the 444us baseline (trace-driven):
  * Q/K projections in fp8e4 DoubleRow (K=256 per instruction, half the PE
    columns). x and Wq/Wk host-quantized with power-of-2 scales; dequant is
    folded into the psum->SBUF bias-add (tensor_scalar mult+add). End-to-end
    rel err ~1.1e-2 (budget 2e-2); V/O/AV stay bf16 - fp8 there measured
    2.7e-2..5.7e-2 offline.
  * Q/K projection for pair t+1 interleaved into the attention loop: the
    attention phase alone is ACT(exp)-bound (12.8us exp vs 11.3us PE per
    pair); adding 6us/pair of proj matmuls keeps the PE the bottleneck and
    the clock ramped.
  * Causal mask identity-matmuls dropped from the PE (-24us): exp writes
    [128c, S) per key-tile (odd tiles' fully-dead first 128 cols never
    computed), the 128-col diagonal triangle gets a 0/1 multiply on DVE, and
    the static zero region [0, 128c) of each persistent ex tile is memset
    once at startup.
  * AV-psum drains, ex gap zeroing and the softmax normalize moved from DVE
    to the idle GPSIMD engine: the per-pair denominator DMA round-trip no
    longer head-of-line-blocks the in-order DVE queue, which was stalling the
    next pair's matmuls on PSUM WAR (7-17us/pair in the baseline trace).
  * PSUM: scores/proj share one [128,1024]x2 pool (4 banks) + 4 AV banks.

Matmul dtype: bf16 (f32r hangs TRN2 hardware - observed empirically); fp8e4
DoubleRow only for Q/K proj.
"""

import os
import sys
import types

import numpy as np

B, S, D, H, HD = 8, 1024, 1024, 16, 64
NT = D // 128          # 8 partition tiles
NP = NT // 2           # 4 DoubleRow pair tiles
PAD_ID = 1.0
SCALE = 1.0 / 8.0      # 1/sqrt(HD)
SX = 32.0              # x fp8 scale (absmax ~4.9 -> 155 < 240)
SW = 4096.0            # Wq/Wk fp8 scale (absmax 1/32 -> 128 < 240)
INV_QK = 1.0 / (SX * SW)

_CACHE = {}
LAST_RESULT = None
LAST_EXEC_NS = None


def _install_trace_hook():
    """Provide antenv.axon_hooks (missing in this image) so trace=True works."""
    try:
        import antenv
        if "antenv.axon_hooks" in sys.modules:
            return True
        m = types.ModuleType("antenv.axon_hooks")
        _hook = [None]
        m.set_axon_ntff_profile_hook = lambda h: _hook.__setitem__(0, h)
        m.get_axon_ntff_profile_hook = lambda: _hook[0]
        sys.modules["antenv.axon_hooks"] = m
        antenv.axon_hooks = m
        from trn_agent_boot.trn_boot import _ntff_profile_via_ctypes
        hook = _ntff_profile_via_ctypes("/opt/axon/libaxon_pjrt.so")
        if hook is None:
            return False
        m.set_axon_ntff_profile_hook(hook)
        return True
    except Exception:
        return False


def _build_graph():
    import concourse.bass as bass
    import concourse.mybir as mybir
    import concourse.tile as tile
    from concourse import bacc

    F32 = mybir.dt.float32
    BF16 = mybir.dt.bfloat16
    F8 = mybir.dt.float8e4
    AluOp = mybir.AluOpType
    Act = mybir.ActivationFunctionType
    DR = mybir.MatmulPerfMode.DoubleRow

    nc = bacc.Bacc(target_bir_lowering=False)

    # fp8 operands, DoubleRow pair-interleaved on host: row block j holds
    # d-rows [256j, 256j+128) as subtile 0 and [256j+128, 256j+256) as 1.
    x8_e = nc.declare_dram_parameter("x8", [NP * 128, 2 * S], F8, isOutput=False)
    wq8_e = nc.declare_dram_parameter("wq8", [NP * 128, 2 * D], F8, isOutput=False)
    wk8_e = nc.declare_dram_parameter("wk8", [NP * 128, 2 * D], F8, isOutput=False)
    xT = nc.declare_dram_parameter("xT", [D, S], BF16, isOutput=False)
    WvT = nc.declare_dram_parameter("WvT", [D, D], BF16, isOutput=False)
    WoT = nc.declare_dram_parameter("WoT", [D, D], BF16, isOutput=False)
    bv = nc.declare_dram_parameter("bv", [D], BF16, isOutput=False)
    bo = nc.declare_dram_parameter("bo", [D], BF16, isOutput=False)
    ones_p = nc.declare_dram_parameter("ones", [S], BF16, isOutput=False)
    # smalls: [128, 24] f32 = ids_r | bq_r | bk_r (each [128, 8], host-packed)
    smalls = nc.declare_dram_parameter("smalls", [128, 3 * NT], F32, isOutput=False)
    # 0/1 causal triangle for the diagonal 128-col window: 1 where col >= row
    tri_p = nc.declare_dram_parameter("tri01", [128, 128], BF16, isOutput=False)
    out_e = nc.declare_dram_parameter("out", [S, D], F32, isOutput=True)

    denom_d = nc.dram_tensor("denom_d", [H, S], BF16)
    recip_d = nc.dram_tensor("recip_d", [H, S], BF16)

    def dep(later, earlier, reason):
        bass._add_dep_helper(later.ins, earlier.ins, reason=reason)

    with tile.TileContext(nc) as tc:
        with tc.tile_pool(name="const", bufs=1) as cp, \
             tc.tile_pool(name="persist", bufs=1) as qp, \
             tc.tile_pool(name="wo", bufs=8) as wop:

            # ---- constants ----
            sm = cp.tile([128, 3 * NT], F32, tag="sm", name="sm")
            nc.sync.dma_start(out=sm[:], in_=smalls[:])
            pad01 = cp.tile([128, NT], F32, tag="pad01", name="pad01")
            nc.vector.tensor_scalar(out=pad01[:], in0=sm[:, 0:NT],
                                    scalar1=PAD_ID, scalar2=None,
                                    op0=AluOp.not_equal)
            bq_col = sm[:, NT:2 * NT]
            bk_col = sm[:, 2 * NT:3 * NT]
            bv_row = cp.tile([1, D], BF16, tag="bvr", name="bv_row")
            nc.sync.dma_start(out=bv_row[:], in_=bv[None, :])
            bo_row = cp.tile([1, D], BF16, tag="bor", name="bo_row")
            nc.sync.dma_start(out=bo_row[:], in_=bo[None, :])
            ones_row = cp.tile([1, S], BF16, tag="ones", name="ones_row")
            nc.sync.dma_start(out=ones_row[:], in_=ones_p[None, :])
            tri = cp.tile([128, 128], BF16, tag="tri", name="tri")
            nc.sync.dma_start(out=tri[:], in_=tri_p[:])

            # fp8 Q/K proj operands (small: 0.25MB + 2x1MB)
            x8_sb = [qp.tile([128, 2, S], F8, tag=f"x8{j}", name=f"x8{j}")
                     for j in range(NP)]
            wq8_sb = [qp.tile([128, 2, D], F8, tag=f"wq8{j}", name=f"wq8{j}")
                      for j in range(NP)]
            wk8_sb = [qp.tile([128, 2, D], F8, tag=f"wk8{j}", name=f"wk8{j}")
                      for j in range(NP)]
            for j in range(NP):
                nc.sync.dma_start(
                    out=x8_sb[j][:], in_=x8_e[j * 128:(j + 1) * 128, :])
            for j in range(NP):
                nc.sync.dma_start(
                    out=wq8_sb[j][:], in_=wq8_e[j * 128:(j + 1) * 128, :])
                nc.sync.dma_start(
                    out=wk8_sb[j][:], in_=wk8_e[j * 128:(j + 1) * 128, :])

            # ---- persistent per-core tensors ----
            Vx = [qp.tile([128, H * (HD + 1)], BF16, tag=f"vx{t}", name=f"vx{t}")
                  for t in range(NT)]
            aoT = [qp.tile([128, S], BF16, tag=f"ao{t}", name=f"ao{t}")
                   for t in range(NT)]
            # persistent exp tiles keyed by (key-tile c, head-in-pair par);
            # [0, 128c) is a static zero region, memset once here.
            exs = {}
            for c in range(NT):
                for par in range(2):
                    ex = qp.tile([128, S], BF16, tag=f"ex{c}_{par}",
                                 name=f"ex{c}_{par}")
                    exs[(c, par)] = ex
                    if c > 0:
                        nc.gpsimd.memset(ex[:, 0:128 * c], 0.0)

            # ============ Phase V: V projection (bf16) ============
            with tc.tile_pool(name="xv", bufs=1) as xp, \
                 tc.tile_pool(name="wst", bufs=8) as wp, \
                 tc.tile_pool(name="psv", bufs=4, space="PSUM") as pvp:

                xT_sb = [xp.tile([128, S], BF16, tag=f"x{c}", name=f"x{c}")
                         for c in range(NT)]
                for c in range(NT):
                    nc.sync.dma_start(out=xT_sb[c][:],
                                      in_=xT[c * 128:(c + 1) * 128, :])
                wv_sb = []
                for c in range(NT):
                    w_t = wp.tile([128, D], BF16, tag="wv", name="wv_t")
                    nc.sync.dma_start(out=w_t[:],
                                      in_=WvT[c * 128:(c + 1) * 128, :])
                    wv_sb.append(w_t)
                for m in range(NT):
                    vdst = Vx[m][:].rearrange("p (h e) -> p h e", e=HD + 1)
                    nc.vector.memset(vdst[:, :, HD:HD + 1], 1.0)
                    for n in range(2):
                        ps = pvp.tile([128, 512], F32, tag="pv", name="pv")
                        for c in range(NT):
                            nc.tensor.matmul(
                                ps[:],
                                xT_sb[c][:, m * 128:(m + 1) * 128],
                                wv_sb[c][:, n * 512:(n + 1) * 512],
                                start=(c == 0), stop=False)
                        nc.tensor.matmul(ps[:], ones_row[:, :128],
                                         bv_row[:, n * 512:(n + 1) * 512],
                                         start=False, stop=True)
                        nc.vector.tensor_copy(
                            out=vdst[:, n * 8:(n + 1) * 8, 0:HD],
                            in_=ps[:].rearrange("p (h e) -> p h e", e=HD))
                    # pad mask: zero whole key rows where ids == PAD,
                    # including the ones column -> denominator excludes them
                    nc.vector.tensor_scalar(
                        out=Vx[m][:], in0=Vx[m][:],
                        scalar1=pad01[:, m:m + 1], scalar2=None,
                        op0=AluOp.mult)

            # ====== Phase A: attention, Q/K proj interleaved per pair ======
            with tc.tile_pool(name="qk", bufs=3) as qkp, \
                 tc.tile_pool(name="stp", bufs=2) as stp, \
                 tc.tile_pool(name="dn", bufs=2) as dnp, \
                 tc.tile_pool(name="rcp", bufs=2) as rcpp, \
                 tc.tile_pool(name="rec", bufs=2) as recp, \
                 tc.tile_pool(name="pssc", bufs=2, space="PSUM") as psc, \
                 tc.tile_pool(name="psav", bufs=1, space="PSUM") as pav:

                # prefetch Wo during attention
                wo_sb = []
                for c in range(NT):
                    w_t = wop.tile([128, D], BF16, tag="wo", name="wo_t")
                    nc.sync.dma_start(out=w_t[:],
                                      in_=WoT[c * 128:(c + 1) * 128, :])
                    wo_sb.append(w_t)

                qt_tiles, kt_tiles = {}, {}

                def qk_proj(m):
                    """fp8 DoubleRow projection of Q/K output tile m."""
                    for w8, dst_map, bias_col, tg in (
                            (wq8_sb, qt_tiles, bq_col, "qt"),
                            (wk8_sb, kt_tiles, bk_col, "kt")):
                        ps = psc.tile([128, 1024], F32, tag="sc", name="ps")
                        for n in range(2):
                            for j in range(NP):
                                nc.tensor.matmul(
                                    ps[:, n * 512:(n + 1) * 512],
                                    w8[j][:, :, m * 128:(m + 1) * 128],
                                    x8_sb[j][:, :, n * 512:(n + 1) * 512],
                                    start=(j == 0), stop=(j == NP - 1),
                                    perf_mode=DR)
                        dst = qkp.tile([128, S], BF16, tag=tg, name=tg)
                        dst_map[m] = dst
                        nc.vector.tensor_scalar(
                            out=dst[:], in0=ps[:],
                            scalar1=INV_QK, scalar2=bias_col[:, m:m + 1],
                            op0=AluOp.mult, op1=AluOp.add)

                qk_proj(0)
                qk_proj(1)

                for t in range(NT):        # head pair (2t, 2t+1)
                    Qt, Kt = qt_tiles[t], kt_tiles[t]
                    av_ps = {(par, g): pav.tile([HD + 1, 512], F32,
                                                tag=f"av{par}{g}",
                                                name=f"av{par}{g}")
                             for par in range(2) for g in range(2)}
                    for c in range(NT):
                        qs = 256 * (c // 2)      # psum tile covers [qs, S)
                        q0 = 128 * c             # first valid query col
                        for par in range(2):
                            base = par * 64
                            sc = psc.tile([128, 1024], F32, tag="sc", name="sc")
                            n0 = q0
                            while n0 < S:
                                n1 = min(qs + 512 * ((n0 - qs) // 512 + 1), S)
                                nc.tensor.matmul(
                                    sc[:, n0 - qs:n1 - qs],
                                    Kt[base:base + 64, c * 128:(c + 1) * 128],
                                    Qt[base:base + 64, n0:n1],
                                    start=True, stop=True)
                                n0 = n1
                            ex = exs[(c, par)]
                            nc.scalar.activation(out=ex[:, q0:S],
                                                 in_=sc[:, q0 - qs:S - qs],
                                                 func=Act.Exp, scale=SCALE)
                            # causal triangle on the diagonal 128-col window
                            nc.vector.tensor_mul(ex[:, q0:q0 + 128],
                                                 ex[:, q0:q0 + 128], tri[:])
                            for g in range(2):
                                if c <= 4 * g + 3:
                                    nc.tensor.matmul(
                                        av_ps[(par, g)][:],
                                        Vx[c][:, (2 * t + par) * (HD + 1):
                                               (2 * t + par + 1) * (HD + 1)],
                                        ex[:, 512 * g:512 * (g + 1)],
                                        start=(c == 0),
                                        stop=(c == min(4 * g + 3, NT - 1)))

                    # epilogue: drain AV psums on GPSIMD (keeps DVE free),
                    # denominator -> reciprocal -> broadcast via DRAM bounce
                    denom_writes = {}
                    for par in range(2):
                        h = 2 * t + par
                        st = stp.tile([HD + 1, S], BF16, tag=f"st{par}",
                                      name=f"st{par}")
                        for g in range(2):
                            nc.scalar.copy(
                                out=st[:, 512 * g:512 * (g + 1)],
                                in_=av_ps[(par, g)][:])
                        nc.sync.dma_start(
                            out=aoT[t][par * 64:par * 64 + HD, :],
                            in_=st[0:HD, :])
                        dw = nc.sync.dma_start(
                            out=denom_d[h:h + 1, :], in_=st[HD:HD + 1, :])
                        denom_writes[h] = dw

                    d_sb = dnp.tile([128, 2, NT], BF16, tag="dsb", name="d_sb")
                    rd = nc.sync.dma_start(
                        out=d_sb[:],
                        in_=denom_d[2 * t:2 * t + 2, :].rearrange(
                            "h (g p) -> p h g", p=128))
                    dep(rd, denom_writes[2 * t], reason="denom RAW via DRAM")
                    dep(rd, denom_writes[2 * t + 1], reason="denom RAW via DRAM")
                    r_sb = rcpp.tile([128, 2, NT], BF16, tag="rsb", name="r_sb")
                    with nc.allow_low_precision(
                            reason="softmax recip in bf16; "
                                   "error budget verified offline"):
                        nc.vector.reciprocal(out=r_sb[:], in_=d_sb[:])
                    wr = nc.sync.dma_start(
                        out=recip_d[2 * t:2 * t + 2, :].rearrange(
                            "h (g p) -> p h g", p=128),
                        in_=r_sb[:])
                    rec = recp.tile([128, S], BF16, tag="rec", name="rec")
                    b0 = nc.sync.dma_start(
                        out=rec[0:64, :],
                        in_=recip_d[2 * t, :][None, :].broadcast_to([64, S]))
                    b1 = nc.sync.dma_start(
                        out=rec[64:128, :],
                        in_=recip_d[2 * t + 1, :][None, :].broadcast_to([64, S]))
                    dep(b0, wr, reason="recip RAW via DRAM")
                    dep(b1, wr, reason="recip RAW via DRAM")
                    nc.gpsimd.tensor_mul(aoT[t][:], aoT[t][:], rec[:])

                    if t + 2 < NT:
                        qk_proj(t + 2)

            # ============ Phase O: output projection ============
            with tc.tile_pool(name="ost", bufs=3) as osp, \
                 tc.tile_pool(name="psf", bufs=4, space="PSUM") as pf:
                for m in range(NT):
                    for n in range(2):
                        ps = pf.tile([128, 512], F32, tag="pf", name="psf")
                        for c in range(NT):
                            nc.tensor.matmul(
                                ps[:],
                                aoT[c][:, m * 128:(m + 1) * 128],
                                wo_sb[c][:, n * 512:(n + 1) * 512],
                                start=(c == 0), stop=False)
                        nc.tensor.matmul(ps[:], ones_row[:, :128],
                                         bo_row[:, n * 512:(n + 1) * 512],
                                         start=False, stop=True)
                        ot = osp.tile([128, 512], F32, tag="ot", name="ot")
                        nc.scalar.copy(out=ot[:], in_=ps[:])
                        nc.sync.dma_start(
                            out=out_e[m * 128:(m + 1) * 128,
                                      n * 512:(n + 1) * 512],
                            in_=ot[:])
    nc.finalize()
    return nc


def _host_consts():
    import ml_dtypes
    bf = ml_dtypes.bfloat16
    jj = np.arange(128)[None, :]
    pp = np.arange(128)[:, None]
    tri01 = (jj >= pp).astype(np.float32).astype(bf)   # 1 where col >= row
    return tri01


def _to_f8(a, scale):
    import ml_dtypes
    return np.asarray(
        np.clip(np.asarray(a, np.float32) * scale, -240.0, 240.0),
        dtype=ml_dtypes.float8_e4m3)


def _pair_interleave(a):
    """[D, N] -> [NP*128, 2*N]: row block j = (d rows 256j..+128 | ..+256)."""
    d, n = a.shape
    return np.ascontiguousarray(
        a.reshape(NP, 2, 128, n).transpose(0, 2, 1, 3).reshape(NP * 128, 2 * n))


def build_in_maps(x, input_ids, Wq, bq, Wk, bk, Wv, bv, Wo, bo):
    import ml_dtypes
    bf = ml_dtypes.bfloat16
    x = np.asarray(x, dtype=np.float32)
    input_ids = np.asarray(input_ids)
    tri01 = _host_consts()
    bq_r = np.ascontiguousarray(np.asarray(bq, np.float32).reshape(NT, 128).T)
    bk_r = np.ascontiguousarray(np.asarray(bk, np.float32).reshape(NT, 128).T)
    wq8 = _pair_interleave(_to_f8(np.asarray(Wq, np.float32).T, SW))
    wk8 = _pair_interleave(_to_f8(np.asarray(Wk, np.float32).T, SW))
    shared = {
        "wq8": wq8, "wk8": wk8,
        "WvT": np.ascontiguousarray(np.asarray(Wv, np.float32).T).astype(bf),
        "WoT": np.ascontiguousarray(np.asarray(Wo, np.float32).T).astype(bf),
        "bv": np.asarray(bv, np.float32).astype(bf),
        "bo": np.asarray(bo, np.float32).astype(bf),
        "ones": np.ones([S], bf),
        "tri01": tri01,
    }
    in_maps = []
    for b in range(B):
        ids_r = input_ids[b].astype(np.float32).reshape(NT, 128).T
        m = dict(shared)
        xb_T = np.ascontiguousarray(x[b].T)
        m["xT"] = xb_T.astype(bf)
        m["x8"] = _pair_interleave(_to_f8(xb_T, SX))
        m["smalls"] = np.ascontiguousarray(
            np.concatenate([ids_r, bq_r, bk_r], axis=1)).astype(np.float32)
        in_maps.append(m)
    return in_maps


def kernel(x, input_ids, Wq, bq, Wk, bk, Wv, bv, Wo, bo):
    global LAST_RESULT, LAST_EXEC_NS
    from concourse.bass_utils import run_bass_kernel_spmd

    if "nc" not in _CACHE:
        _CACHE["nc"] = _build_graph()
    nc = _CACHE["nc"]
    in_maps = build_in_maps(x, input_ids, Wq, bq, Wk, bk, Wv, bv, Wo, bo)

    trace = os.environ.get("KERNEL_TRACE", "0") == "1" and _install_trace_hook()
    res = run_bass_kernel_spmd(nc, in_maps, core_ids=list(range(B)), trace=trace)
    LAST_RESULT = res
    LAST_EXEC_NS = res.exec_time_ns
    return np.stack([res.results[b]["out"] for b in range(B)]).astype(np.float32)
